# revision 23
# baseline (speedup 1.0000x reference)
"""Trainium2 Bass kernel for nn_AttentionLayer (B=8, S=2048, EMB=512, FF=64).

Strategy: data-parallel over batch — each of the 8 NeuronCores processes one
batch element independently (no collectives). The whole per-core computation
runs in a feature-major ("transposed") layout so that every matmul contraction
lands on the partition dimension and no on-device transposes are needed:

  Q^T = (Wq/sqrt(d)) @ query^T + bq/sqrt(d)      [d, s]   (bias per-partition)
  K^T = Wk @ key^T + bk                          [d, s]
  V   = value @ Wv^T + bv                        [s, d]   (bias via ones-row MM)
  S^T[k,q] = sum_d K^T[d,k] Q^T[d,q]             (scores, pre-scaled)
  E^T = exp(S^T) * maskT                         (no max-subtraction needed:
                                                  |scores| <~ 2, exp is safe)
  U^T[d,q] = sum_k V[k,d] E^T[k,q]               (unnormalized attention)
  rowsum[q] = sum_k E^T[k,q]                     (ones-column matmul)
  x1 = query^T + U^T / rowsum
  out1 = LN1(x1)   (stats over d = partition dim via ones matmuls)
  h^T = relu(W1 @ out1^T + b1);  ff^T = W2 @ h^T + b2 (bias via ones row)
  out^T = LN2(out1 + ff)                         -> host transposes back
"""

import sys

if "/opt/trn_rl_repo" not in sys.path:
    sys.path.insert(0, "/opt/trn_rl_repo")

import numpy as np

import concourse.bass as bass
import concourse.bacc as bacc
import concourse.tile as tile
from concourse import mybir
from concourse.bass_utils import run_bass_kernel_spmd

P = 128
S = 2048
D = 512
FF = 64
B = 8
CH = D // P          # 4 chunks of the emb dim
KT = S // P          # 16 key tiles
NB = 512             # q-block width (matmul free dim / PSUM bank)
QB = S // NB         # 4 q-blocks
EPS = 1e-5
SCALE = 1.0 / np.sqrt(np.float32(D))

F32 = mybir.dt.float32
BF16 = mybir.dt.bfloat16
F32R = mybir.dt.float32r
AF = mybir.ActivationFunctionType
OP = mybir.AluOpType

NPBF16 = mybir.dt.np(BF16)


from contextlib import ExitStack, contextmanager


@contextmanager
def TileCtx(nc):
    with tile.TileContext(nc) as tc:
        with ExitStack() as es:
            yield tc, es


def build():
    nc = bacc.Bacc(
        "TRN2", target_bir_lowering=False, debug=False, num_devices=B
    )

    d_qT = nc.dram_tensor("qT", [D, S], F32, kind="ExternalInput")
    d_qTb = nc.dram_tensor("qTb", [D, S], BF16, kind="ExternalInput")
    d_kTb = nc.dram_tensor("kTb", [D, S], BF16, kind="ExternalInput")
    d_vTb = nc.dram_tensor("vTb", [D, S], BF16, kind="ExternalInput")
    d_maskT = nc.dram_tensor("maskT", [S, S], BF16, kind="ExternalInput")
    d_wq = nc.dram_tensor("wq", [P, CH, D], BF16, kind="ExternalInput")
    d_wk = nc.dram_tensor("wk", [P, CH, D], BF16, kind="ExternalInput")
    d_wv = nc.dram_tensor("wv", [P, CH, D], BF16, kind="ExternalInput")
    d_w1 = nc.dram_tensor("w1", [P, CH, FF], BF16, kind="ExternalInput")
    d_w2b = nc.dram_tensor("w2b", [FF + 1, D], BF16, kind="ExternalInput")
    d_bq = nc.dram_tensor("bq", [P, CH], F32, kind="ExternalInput")
    d_bk = nc.dram_tensor("bk", [P, CH], F32, kind="ExternalInput")
    d_bv = nc.dram_tensor("bv", [1, D], BF16, kind="ExternalInput")
    d_b1 = nc.dram_tensor("b1", [FF, 1], F32, kind="ExternalInput")
    d_g1c = nc.dram_tensor("g1c", [P, CH], F32, kind="ExternalInput")
    d_be1c = nc.dram_tensor("be1c", [P, CH], F32, kind="ExternalInput")
    d_g2c = nc.dram_tensor("g2c", [P, CH], F32, kind="ExternalInput")
    d_be2c = nc.dram_tensor("be2c", [P, CH], F32, kind="ExternalInput")
    d_g1r = nc.dram_tensor("g1r", [1, D], BF16, kind="ExternalInput")
    d_g2r = nc.dram_tensor("g2r", [1, D], BF16, kind="ExternalInput")
    d_outT = nc.dram_tensor("outT", [D, S], F32, kind="ExternalOutput")

    qT3 = d_qT.rearrange("(c p) s -> p c s", p=P)
    qTb3 = d_qTb.rearrange("(c p) s -> p c s", p=P)
    kTb3 = d_kTb.rearrange("(c p) s -> p c s", p=P)
    vTb3 = d_vTb.rearrange("(c p) s -> p c s", p=P)
    maskT3 = d_maskT.rearrange("(t p) s -> p t s", p=P)
    outT3 = d_outT.rearrange("(c p) s -> p c s", p=P)

    from contextlib import ExitStack

    with TileCtx(nc) as (tc, es):
            cpool = es.enter_context(tc.tile_pool(name="const", bufs=1))
            xf = es.enter_context(tc.tile_pool(name="xf", bufs=2))
            ofp = es.enter_context(tc.tile_pool(name="ofp", bufs=2))
            xb = es.enter_context(tc.tile_pool(name="xb", bufs=5))
            qkv = es.enter_context(tc.tile_pool(name="qkv", bufs=1))
            epool = es.enter_context(tc.tile_pool(name="epool", bufs=18))
            mpool = es.enter_context(tc.tile_pool(name="mpool", bufs=3))
            o1pool = es.enter_context(tc.tile_pool(name="o1pool", bufs=6))
            x1pool = es.enter_context(tc.tile_pool(name="x1pool", bufs=5))
            sqpool = es.enter_context(tc.tile_pool(name="sqpool", bufs=4))
            x2pool = es.enter_context(tc.tile_pool(name="x2pool", bufs=5))
            hpool = es.enter_context(tc.tile_pool(name="hpool", bufs=2))
            rbpool = es.enter_context(tc.tile_pool(name="rbpool", bufs=2))
            rows = es.enter_context(tc.tile_pool(name="rows", bufs=6))
            pa = es.enter_context(tc.tile_pool(name="pa", bufs=2, space="PSUM"))
            pb = es.enter_context(tc.tile_pool(name="pb", bufs=2, space="PSUM"))
            prow = es.enter_context(tc.tile_pool(name="prow", bufs=1, space="PSUM"))
            pm = es.enter_context(tc.tile_pool(name="pm", bufs=3, space="PSUM"))
            # ---------------- constants ----------------
            wq_sb = cpool.tile([P, CH, D], BF16, name="wq_sb")
            wk_sb = cpool.tile([P, CH, D], BF16, name="wk_sb")
            wv_sb = cpool.tile([P, CH, D], BF16, name="wv_sb")
            w1_sb = cpool.tile([P, CH, FF], BF16, name="w1_sb")
            w2_sb = cpool.tile([FF + 1, D], BF16, name="w2_sb")
            bq_sb = cpool.tile([P, CH], F32, name="bq_sb")
            bk_sb = cpool.tile([P, CH], F32, name="bk_sb")
            bv_sb = cpool.tile([1, D], BF16, name="bv_sb")
            b1_sb = cpool.tile([FF, 1], F32, name="b1_sb")
            g1c_sb = cpool.tile([P, CH], F32, name="g1c_sb")
            be1c_sb = cpool.tile([P, CH], F32, name="be1c_sb")
            g2c_sb = cpool.tile([P, CH], F32, name="g2c_sb")
            be2c_sb = cpool.tile([P, CH], F32, name="be2c_sb")
            g1r_sb = cpool.tile([1, D], BF16, name="g1r_sb")
            g2r_sb = cpool.tile([1, D], BF16, name="g2r_sb")
            nc.sync.dma_start(out=wq_sb, in_=d_wq[:])
            nc.sync.dma_start(out=wk_sb, in_=d_wk[:])
            nc.sync.dma_start(out=wv_sb, in_=d_wv[:])
            nc.sync.dma_start(out=w1_sb, in_=d_w1[:])
            nc.sync.dma_start(out=w2_sb, in_=d_w2b[:])
            nc.sync.dma_start(out=bq_sb, in_=d_bq[:])
            nc.sync.dma_start(out=bk_sb, in_=d_bk[:])
            nc.sync.dma_start(out=bv_sb, in_=d_bv[:])
            nc.sync.dma_start(out=b1_sb, in_=d_b1[:])
            nc.sync.dma_start(out=g1c_sb, in_=d_g1c[:])
            nc.sync.dma_start(out=be1c_sb, in_=d_be1c[:])
            nc.sync.dma_start(out=g2c_sb, in_=d_g2c[:])
            nc.sync.dma_start(out=be2c_sb, in_=d_be2c[:])
            nc.sync.dma_start(out=g1r_sb, in_=d_g1r[:])
            nc.sync.dma_start(out=g2r_sb, in_=d_g2r[:])

            ones_col_b = cpool.tile([P, 1], BF16, name="ones_col_b")
            ones_row_b = cpool.tile([1, P], BF16, name="ones_row_b")
            eps_sb = cpool.tile([1, 1], F32, name="eps_sb")
            nc.vector.memset(ones_col_b, 1.0)
            nc.vector.memset(ones_row_b, 1.0)
            nc.vector.memset(eps_sb, EPS)

            # ---------------- phase A: projections ----------------
            def load_bf16(dram3, name):
                chunks = []
                for c in range(CH):
                    xbc = xb.tile([P, S], BF16, tag="xb", name=f"{name}b{c}")
                    nc.sync.dma_start(out=xbc, in_=dram3[:, c, :])
                    chunks.append(xbc)
                return chunks

            qTb = load_bf16(qTb3, "q")
            # Q^T and K^T, feature-major with per-partition bias.
            QT = [qkv.tile([P, S], BF16, name=f"QT{c}") for c in range(CH)]
            KT_sb = [qkv.tile([P, S], BF16, name=f"KTs{c}") for c in range(CH)]
            for fc in range(CH):
                for j in range(QB):
                    ps = pa.tile([P, NB], F32, tag="pa", name="ps_q")
                    for c in range(CH):
                        nc.tensor.matmul(
                            ps,
                            wq_sb[:, c, fc * P:(fc + 1) * P],
                            qTb[c][:, j * NB:(j + 1) * NB],
                            start=(c == 0),
                            stop=(c == CH - 1),
                        )
                    nc.scalar.activation(
                        QT[fc][:, j * NB:(j + 1) * NB], ps, AF.Identity,
                        bias=bq_sb[:, fc:fc + 1], scale=1.0,
                    )
            kTb = load_bf16(kTb3, "k")
            for fc in range(CH):
                for j in range(QB):
                    ps = pa.tile([P, NB], F32, tag="pa", name="ps_k")
                    for c in range(CH):
                        nc.tensor.matmul(
                            ps,
                            wk_sb[:, c, fc * P:(fc + 1) * P],
                            kTb[c][:, j * NB:(j + 1) * NB],
                            start=(c == 0),
                            stop=(c == CH - 1),
                        )
                    nc.scalar.activation(
                        KT_sb[fc][:, j * NB:(j + 1) * NB], ps, AF.Identity,
                        bias=bk_sb[:, fc:fc + 1], scale=1.0,
                    )
            vTb = load_bf16(vTb3, "v")
            # V in natural [s, d] layout; bias added via ones-row matmul.
            V_sb = [qkv.tile([P, D], BF16, name=f"V{t}") for t in range(KT)]
            for t in range(KT):
                ps = pa.tile([P, D], F32, tag="pa", name="ps_v")
                for c in range(CH):
                    nc.tensor.matmul(
                        ps,
                        vTb[c][:, t * P:(t + 1) * P],
                        wv_sb[:, c, :],
                        start=(c == 0),
                        stop=False,
                    )
                nc.tensor.matmul(ps, ones_row_b, bv_sb, start=False, stop=True)
                nc.scalar.copy(V_sb[t], ps)

            # ---------------- phase B: attention + FFN per q-block ----------------
            for j in range(QB):
                jq = slice(j * NB, (j + 1) * NB)

                mtiles = []
                for g in range(4):
                    mt = mpool.tile([P, 4, NB], BF16, tag="m", name="mt")
                    nc.sync.dma_start(out=mt, in_=maskT3[:, 4 * g:4 * g + 4, jq])
                    mtiles.append(mt)
                qres = xf.tile([P, CH, NB], F32, tag="xf", name="qres")
                nc.sync.dma_start(out=qres, in_=qT3[:, :, jq])

                # scores -> exp*mask -> attention (d-chunks 0,1) + rowsum
                U01 = [pb.tile([P, NB], F32, tag="u", name="u01") for _ in range(2)]
                rsum = prow.tile([1, NB], F32, name="rsum")
                estrips = []
                for kt in range(KT):
                    sc = pa.tile([P, NB], F32, tag="pa", name="sc")
                    for c in range(CH):
                        nc.tensor.matmul(
                            sc,
                            KT_sb[c][:, kt * P:(kt + 1) * P],
                            QT[c][:, jq],
                            start=(c == 0),
                            stop=(c == CH - 1),
                        )
                    e = epool.tile([P, NB], BF16, tag="e", name="e")
                    nc.scalar.activation(e, sc, AF.Exp)
                    nc.vector.tensor_mul(e, e, mtiles[kt // 4][:, kt % 4, :])
                    estrips.append(e)
                    for c in range(2):
                        nc.tensor.matmul(
                            U01[c],
                            V_sb[kt][:, c * P:(c + 1) * P],
                            e,
                            start=(kt == 0),
                            stop=(kt == KT - 1),
                        )
                    nc.tensor.matmul(
                        rsum, ones_col_b, e, start=(kt == 0), stop=(kt == KT - 1)
                    )

                # reciprocal of rowsum, broadcast to [128, NB]
                rs_row = rows.tile([1, NB], F32, tag="r", name="rs_row")
                nc.scalar.copy(rs_row, rsum)
                nc.vector.reciprocal(rs_row, rs_row)
                rs_row_b = rows.tile([1, NB], BF16, tag="rb16", name="rs_row_b", bufs=4)
                nc.gpsimd.tensor_copy(out=rs_row_b, in_=rs_row)
                rb_ps = pm.tile([P, NB], F32, tag="m", name="rb_ps")
                nc.tensor.matmul(rb_ps, ones_row_b, rs_row_b,
                                 start=True, stop=True)
                recip_b = rbpool.tile([P, NB], F32, tag="rb", name="recip_b")
                nc.scalar.copy(recip_b, rb_ps)

                # x1 = queryT + U^T * recip  (chunks 0,1 now; 2,3 after pass 2)
                x1 = [None] * CH
                for c in range(2):
                    x1[c] = x1pool.tile([P, NB], BF16, tag="x1", name="x1")
                    nc.vector.tensor_mul(x1[c], U01[c], recip_b)
                    nc.vector.tensor_add(x1[c], x1[c], qres[:, c, :])
                U23 = [pb.tile([P, NB], F32, tag="u", name="u23") for _ in range(2)]
                for kt in range(KT):
                    for c in range(2):
                        nc.tensor.matmul(
                            U23[c],
                            V_sb[kt][:, (c + 2) * P:(c + 3) * P],
                            estrips[kt],
                            start=(kt == 0),
                            stop=(kt == KT - 1),
                        )
                for c in range(2, CH):
                    x1[c] = x1pool.tile([P, NB], BF16, tag="x1", name="x1")
                    nc.vector.tensor_mul(x1[c], U23[c - 2], recip_b)
                    nc.vector.tensor_add(x1[c], x1[c], qres[:, c, :])

                def layer_norm(xc, gc_sb, bc_sb, gr_sb, out_tiles, out_slices):
                    """Partition-dim layernorm over the CH chunks of xc (f32).

                    Writes gamma*(x-mu)*rstd+beta into out_tiles[c][out_slices[c]].
                    """
                    s1 = pm.tile([P, NB], F32, tag="m", name="s1")
                    for c in range(CH):
                        nc.tensor.matmul(
                            s1[0:1, :], ones_col_b, xc[c],
                            start=(c == 0), stop=(c == CH - 1),
                        )
                    sq = [sqpool.tile([P, NB], BF16, tag="sq", name="sq")
                          for _ in range(CH)]
                    for c in range(CH):
                        nc.scalar.activation(sq[c], xc[c], AF.Square)
                    s2 = pm.tile([P, NB], F32, tag="m", name="s2")
                    for c in range(CH):
                        nc.tensor.matmul(
                            s2[0:1, :], ones_col_b, sq[c],
                            start=(c == 0), stop=(c == CH - 1),
                        )
                    mu = rows.tile([1, NB], F32, tag="r", name="mu")
                    nc.scalar.mul(mu, s1[0:1, :], 1.0 / D)
                    msq = rows.tile([1, NB], F32, tag="r", name="msq")
                    nc.vector.tensor_mul(msq, mu, mu)
                    var = rows.tile([1, NB], F32, tag="r", name="var")
                    nc.scalar.mul(var, s2[0:1, :], 1.0 / D)
                    nc.vector.tensor_sub(var, var, msq)
                    nc.scalar.activation(var, var, AF.Sqrt, bias=eps_sb)
                    rstd = rows.tile([1, NB], F32, tag="r", name="rstd")
                    nc.vector.reciprocal(rstd, var)
                    rstd_b16 = rows.tile([1, NB], BF16, tag="rb16", name="rstd_b16", bufs=4)
                    nc.gpsimd.tensor_copy(out=rstd_b16, in_=rstd)
                    mur = rows.tile([1, NB], BF16, tag="rb16", name="mur", bufs=4)
                    nc.vector.tensor_mul(mur, mu, rstd)
                    # broadcast rstd along partitions; outer product gamma x mur
                    rstd_b = pm.tile([P, NB], F32, tag="m", name="rstd_b")
                    nc.tensor.matmul(rstd_b, ones_row_b, rstd_b16,
                                     start=True, stop=True)
                    for c in range(CH):
                        mg_b = pm.tile([P, NB], F32, tag="m", name="mg_b")
                        nc.tensor.matmul(
                            mg_b, gr_sb[:, c * P:(c + 1) * P], mur,
                            start=True, stop=True,
                        )
                        # t = (x*gamma) * rstd_b ; out = (t + beta) - gamma*mur
                        t = sqpool.tile([P, NB], F32, tag="t", name="t")
                        nc.vector.scalar_tensor_tensor(
                            t, xc[c], gc_sb[:, c:c + 1], rstd_b,
                            op0=OP.mult, op1=OP.mult,
                        )
                        nc.vector.scalar_tensor_tensor(
                            out_tiles[c][out_slices[c]], t, bc_sb[:, c:c + 1],
                            mg_b, op0=OP.add, op1=OP.subtract,
                        )

                out1 = [o1pool.tile([P, NB], BF16, tag="o1", name="out1")
                        for _ in range(CH)]
                layer_norm(x1, g1c_sb, be1c_sb, g1r_sb, out1,
                           [np.s_[:, :]] * CH)

                # FFN: h = relu(W1 @ out1^T + b1), with ones row for W2 bias
                hp = pm.tile([P, NB], F32, tag="m", name="hp")
                for c in range(CH):
                    nc.tensor.matmul(
                        hp[0:FF, :], w1_sb[:, c, :], out1[c],
                        start=(c == 0), stop=(c == CH - 1),
                    )
                h = hpool.tile([FF + 1, NB], BF16, tag="h", name="h")
                nc.scalar.activation(h[0:FF, :], hp[0:FF, :], AF.Relu, bias=b1_sb)
                nc.vector.memset(h[FF:FF + 1, :], 1.0)
                x2 = []
                for c in range(CH):
                    fp = pm.tile([P, NB], F32, tag="m", name="fp")
                    nc.tensor.matmul(
                        fp, w2_sb[:, c * P:(c + 1) * P], h, start=True, stop=True
                    )
                    x2c = x2pool.tile([P, NB], BF16, tag="x2", name="x2c")
                    nc.vector.tensor_add(x2c, fp, out1[c])
                    x2.append(x2c)

                ofin = ofp.tile([P, CH, NB], F32, tag="of", name="ofin")
                layer_norm(x2, g2c_sb, be2c_sb, g2r_sb,
                           [ofin] * CH,
                           [np.s_[:, c, :] for c in range(CH)])
                nc.sync.dma_start(out=outT3[:, :, jq], in_=ofin)

    nc.finalize()
    return nc


_NC = None


def _get_nc():
    global _NC
    if _NC is None:
        _NC = build()
    return _NC


def _stage_weights(Wq, bq, Wk, bk, Wv, bv, g1, be1, g2, be2, W1, b1, W2, b2):
    def chunked_T(w):  # [f, e] weight -> [p, c, f] with partition = e within chunk
        return np.ascontiguousarray(
            w.T.reshape(CH, P, -1).transpose(1, 0, 2)
        )

    def col(v):  # [D] -> [p, c]
        return np.ascontiguousarray(v.reshape(CH, P).T)

    return {
        "wq": chunked_T(Wq.astype(np.float32) * SCALE).astype(NPBF16),
        "wk": chunked_T(Wk).astype(NPBF16),
        "wv": chunked_T(Wv).astype(NPBF16),
        "w1": chunked_T(W1).astype(NPBF16),
        "w2b": np.ascontiguousarray(
            np.concatenate([W2.T, b2[None, :]], axis=0)
        ).astype(NPBF16),
        "bq": col(bq.astype(np.float32) * SCALE),
        "bk": col(bk),
        "bv": np.ascontiguousarray(bv[None, :]).astype(NPBF16),
        "b1": np.ascontiguousarray(b1[:, None]).astype(np.float32),
        "g1c": col(g1),
        "be1c": col(be1),
        "g2c": col(g2),
        "be2c": col(be2),
        "g1r": np.ascontiguousarray(g1[None, :]).astype(NPBF16),
        "g2r": np.ascontiguousarray(g2[None, :]).astype(NPBF16),
    }


def run(inputs, trace=False, **kwargs):
    """Run on the 8 NeuronCores; returns (output [B,S,D] f32, BassKernelResults)."""
    nc = _get_nc()
    w = _stage_weights(
        inputs["Wq"], inputs["bq"], inputs["Wk"], inputs["bk"], inputs["Wv"],
        inputs["bv"], inputs["g1"], inputs["be1"], inputs["g2"], inputs["be2"],
        inputs["W1"], inputs["b1"], inputs["W2"], inputs["b2"],
    )
    w = {k: np.asarray(v) for k, v in w.items()}
    query = np.asarray(inputs["query"], np.float32)
    key = np.asarray(inputs["key"], np.float32)
    value = np.asarray(inputs["value"], np.float32)
    mask = np.asarray(inputs["mask"])
    in_maps = []
    for b in range(B):
        m = dict(w)
        qt = np.ascontiguousarray(query[b].T)
        m["qT"] = qt
        m["qTb"] = qt.astype(NPBF16)
        m["kTb"] = np.ascontiguousarray(key[b].T).astype(NPBF16)
        m["vTb"] = np.ascontiguousarray(value[b].T).astype(NPBF16)
        m["maskT"] = np.ascontiguousarray(mask[b].T).astype(NPBF16)
        in_maps.append(m)
    res = run_bass_kernel_spmd(nc, in_maps, core_ids=list(range(B)),
                               trace=trace, **kwargs)
    out = np.stack(
        [np.asarray(res.results[b]["outT"], np.float32).T for b in range(B)]
    )
    return out, res


def kernel(**inputs) -> np.ndarray:
    out, _ = run(inputs)
    return out


# revision 32
# speedup vs baseline: 1.5631x; 1.5631x over previous
"""Trainium2 Bass kernel for nn_AttentionLayer (B=8, S=2048, EMB=512, FF=64).

Strategy: data-parallel over batch — each of the 8 NeuronCores processes one
batch element independently (no collectives). The whole per-core computation
runs in a feature-major ("transposed") layout so that every matmul contraction
lands on the partition dimension and no on-device transposes are needed:

  Q^T = (Wq/sqrt(d)) @ query^T + bq/sqrt(d)      [d, s]   (bias per-partition)
  K^T = Wk @ key^T + bk                          [d, s]
  V   = value @ Wv^T + bv                        [s, d]   (bias via ones-row MM)
  S^T[k,q] = sum_d K^T[d,k] Q^T[d,q]             (scores, pre-scaled)
  E^T = exp(S^T) * maskT                         (no max-subtraction needed:
                                                  |scores| <~ 2, exp is safe)
  U^T[d,q] = sum_k V[k,d] E^T[k,q]               (unnormalized attention)
  rowsum[q] = sum_k E^T[k,q]                     (ones-column matmul)
  x1 = query^T + U^T / rowsum
  out1 = LN1(x1)   (stats over d = partition dim via ones matmuls)
  h^T = relu(W1 @ out1^T + b1);  ff^T = W2 @ h^T + b2 (bias via ones row)
  out^T = LN2(out1 + ff)                         -> host transposes back
"""

import sys

if "/opt/trn_rl_repo" not in sys.path:
    sys.path.insert(0, "/opt/trn_rl_repo")

import numpy as np

import concourse.bass as bass
import concourse.bacc as bacc
import concourse.tile as tile
from concourse import mybir
from concourse.bass_utils import run_bass_kernel_spmd

P = 128
S = 2048
D = 512
FF = 64
B = 8
CH = D // P          # 4 chunks of the emb dim
KT = S // P          # 16 key tiles
NB = 512             # q-block width (matmul free dim / PSUM bank)
QB = S // NB         # 4 q-blocks
EPS = 1e-5
SCALE = 1.0 / np.sqrt(np.float32(D))

F32 = mybir.dt.float32
BF16 = mybir.dt.bfloat16
F32R = mybir.dt.float32r
AF = mybir.ActivationFunctionType
OP = mybir.AluOpType

NPBF16 = mybir.dt.np(BF16)


from contextlib import ExitStack, contextmanager


@contextmanager
def TileCtx(nc):
    with tile.TileContext(nc) as tc:
        with ExitStack() as es:
            yield tc, es


def build():
    nc = bacc.Bacc(
        "TRN2", target_bir_lowering=False, debug=False, num_devices=B
    )

    d_qT = nc.dram_tensor("qT", [D, S], F32, kind="ExternalInput")
    d_qTb = nc.dram_tensor("qTb", [D, S], BF16, kind="ExternalInput")
    d_kTb = nc.dram_tensor("kTb", [D, S], BF16, kind="ExternalInput")
    d_vTb = nc.dram_tensor("vTb", [D, S], BF16, kind="ExternalInput")
    d_maskT = nc.dram_tensor("maskT", [S, S], BF16, kind="ExternalInput")
    d_wq = nc.dram_tensor("wq", [P, CH, D], BF16, kind="ExternalInput")
    d_wk = nc.dram_tensor("wk", [P, CH, D], BF16, kind="ExternalInput")
    d_wv = nc.dram_tensor("wv", [P, CH, D], BF16, kind="ExternalInput")
    d_w1 = nc.dram_tensor("w1", [P, CH, FF], BF16, kind="ExternalInput")
    d_w2b = nc.dram_tensor("w2b", [FF + 1, D], BF16, kind="ExternalInput")
    d_bq = nc.dram_tensor("bq", [P, CH], F32, kind="ExternalInput")
    d_bk = nc.dram_tensor("bk", [P, CH], F32, kind="ExternalInput")
    d_bv = nc.dram_tensor("bv", [P, CH], F32, kind="ExternalInput")
    d_b1 = nc.dram_tensor("b1", [FF, 1], F32, kind="ExternalInput")
    d_g1c = nc.dram_tensor("g1c", [P, CH], F32, kind="ExternalInput")
    d_be1c = nc.dram_tensor("be1c", [P, CH], F32, kind="ExternalInput")
    d_g2c = nc.dram_tensor("g2c", [P, CH], F32, kind="ExternalInput")
    d_be2c = nc.dram_tensor("be2c", [P, CH], F32, kind="ExternalInput")
    d_g1r = nc.dram_tensor("g1r", [1, D], BF16, kind="ExternalInput")
    d_g2r = nc.dram_tensor("g2r", [1, D], BF16, kind="ExternalInput")
    d_outT = nc.dram_tensor("outT", [D, S], F32, kind="ExternalOutput")

    qT3 = d_qT.rearrange("(c p) s -> p c s", p=P)
    qTb3 = d_qTb.rearrange("(c p) s -> p c s", p=P)
    kTb3 = d_kTb.rearrange("(c p) s -> p c s", p=P)
    vTb3 = d_vTb.rearrange("(c p) s -> p c s", p=P)
    maskT3 = d_maskT.rearrange("(t p) s -> p t s", p=P)
    outT3 = d_outT.rearrange("(c p) s -> p c s", p=P)

    from contextlib import ExitStack

    with TileCtx(nc) as (tc, es):
            cpool = es.enter_context(tc.tile_pool(name="const", bufs=1))
            xf = es.enter_context(tc.tile_pool(name="xf", bufs=2))
            ofp = es.enter_context(tc.tile_pool(name="ofp", bufs=2))
            xb = es.enter_context(tc.tile_pool(name="xb", bufs=11))
            usb = es.enter_context(tc.tile_pool(name="usb", bufs=4))
            qkv = es.enter_context(tc.tile_pool(name="qkv", bufs=1))
            epool = es.enter_context(tc.tile_pool(name="epool", bufs=17))
            mpool = es.enter_context(tc.tile_pool(name="mpool", bufs=3))
            o1pool = es.enter_context(tc.tile_pool(name="o1pool", bufs=8))
            x1pool = es.enter_context(tc.tile_pool(name="x1pool", bufs=8))
            sqpool = es.enter_context(tc.tile_pool(name="sqpool", bufs=6))
            x2pool = es.enter_context(tc.tile_pool(name="x2pool", bufs=8))
            hpool = es.enter_context(tc.tile_pool(name="hpool", bufs=2))
            rbpool = es.enter_context(tc.tile_pool(name="rbpool", bufs=2))
            rows = es.enter_context(tc.tile_pool(name="rows", bufs=5))
            pa = es.enter_context(tc.tile_pool(name="pa", bufs=2, space="PSUM"))
            pb = es.enter_context(tc.tile_pool(name="pb", bufs=2, space="PSUM"))
            prow = es.enter_context(tc.tile_pool(name="prow", bufs=1, space="PSUM"))
            pm = es.enter_context(tc.tile_pool(name="pm", bufs=3, space="PSUM"))
            # ---------------- constants ----------------
            wq_sb = cpool.tile([P, CH, D], BF16, name="wq_sb")
            wk_sb = cpool.tile([P, CH, D], BF16, name="wk_sb")
            wv_sb = cpool.tile([P, CH, D], BF16, name="wv_sb")
            w1_sb = cpool.tile([P, CH, FF], BF16, name="w1_sb")
            w2_sb = cpool.tile([FF + 1, D], BF16, name="w2_sb")
            bq_sb = cpool.tile([P, CH], F32, name="bq_sb")
            bk_sb = cpool.tile([P, CH], F32, name="bk_sb")
            bv_sb = cpool.tile([P, CH], F32, name="bv_sb")
            b1_sb = cpool.tile([FF, 1], F32, name="b1_sb")
            g1c_sb = cpool.tile([P, CH], F32, name="g1c_sb")
            be1c_sb = cpool.tile([P, CH], F32, name="be1c_sb")
            g2c_sb = cpool.tile([P, CH], F32, name="g2c_sb")
            be2c_sb = cpool.tile([P, CH], F32, name="be2c_sb")
            g1r_sb = cpool.tile([1, D], BF16, name="g1r_sb")
            g2r_sb = cpool.tile([1, D], BF16, name="g2r_sb")
            # weights/biases on the critical path load on the sync queue,
            # interleaved with the input halves (emitted in load_halves below);
            # everything needed only later goes through the idle gpsimd queue.
            nc.gpsimd.dma_start(out=w1_sb, in_=d_w1[:])
            nc.gpsimd.dma_start(out=w2_sb, in_=d_w2b[:])
            nc.gpsimd.dma_start(out=bv_sb, in_=d_bv[:])
            nc.gpsimd.dma_start(out=b1_sb, in_=d_b1[:])
            nc.gpsimd.dma_start(out=g1c_sb, in_=d_g1c[:])
            nc.gpsimd.dma_start(out=be1c_sb, in_=d_be1c[:])
            nc.gpsimd.dma_start(out=g2c_sb, in_=d_g2c[:])
            nc.gpsimd.dma_start(out=be2c_sb, in_=d_be2c[:])
            nc.gpsimd.dma_start(out=g1r_sb, in_=d_g1r[:])
            nc.gpsimd.dma_start(out=g2r_sb, in_=d_g2r[:])

            ones_col_b = cpool.tile([P, 1], BF16, name="ones_col_b")
            ones_row_b = cpool.tile([1, P], BF16, name="ones_row_b")
            eps_sb = cpool.tile([1, 1], F32, name="eps_sb")
            nc.vector.memset(ones_col_b, 1.0)
            nc.vector.memset(ones_row_b, 1.0)
            nc.vector.memset(eps_sb, EPS)

            # ---------------- phase A: projections ----------------
            # Inputs stream in as half-chunks [128, 1024] so the first
            # projection group is ready after ~1.5 MB of DMA, not 4 MB.
            HW_ = S // 2

            def load_tensor_priority(w_tile, d_w, b_tile, d_b, dram3, name):
                nc.sync.dma_start(out=w_tile, in_=d_w[:])
                halves = [[None, None] for _ in range(CH)]
                for c in range(CH):
                    xt = xb.tile([P, HW_], BF16, tag="xb", name=f"{name}{c}_0")
                    nc.sync.dma_start(out=xt, in_=dram3[:, c, 0:HW_])
                    halves[c][0] = xt
                if b_tile is not None:
                    nc.sync.dma_start(out=b_tile, in_=d_b[:])
                for c in range(CH):
                    xt = xb.tile([P, HW_], BF16, tag="xb", name=f"{name}{c}_1")
                    nc.sync.dma_start(out=xt, in_=dram3[:, c, HW_:S])
                    halves[c][1] = xt
                return halves

            qTh = load_tensor_priority(wq_sb, d_wq, bq_sb, d_bq, qTb3, "qh")
            kTh = load_tensor_priority(wk_sb, d_wk, bk_sb, d_bk, kTb3, "kh")
            vTh = load_tensor_priority(wv_sb, d_wv, None, None, vTb3, "vh")

            QT = [qkv.tile([P, S], BF16, name=f"QT{c}") for c in range(CH)]
            KT_sb = [qkv.tile([P, S], BF16, name=f"KTs{c}") for c in range(CH)]
            V_sb = [qkv.tile([P, D], BF16, name=f"V{t}") for t in range(KT)]

            def proj_T(w_sb, xh, bias_sb, out_tiles):
                for j in range(QB):
                    hh, loc = j // 2, (j % 2) * NB
                    for fc in range(CH):
                        ps = pa.tile([P, NB], F32, tag="pa", name="ps")
                        for c in range(CH):
                            nc.tensor.matmul(
                                ps,
                                w_sb[:, c, fc * P:(fc + 1) * P],
                                xh[c][hh][:, loc:loc + NB],
                                start=(c == 0),
                                stop=(c == CH - 1),
                            )
                        nc.scalar.activation(
                            out_tiles[fc][:, j * NB:(j + 1) * NB], ps,
                            AF.Identity, bias=bias_sb[:, fc:fc + 1], scale=1.0,
                        )

            proj_T(wq_sb, qTh, bq_sb, QT)
            proj_T(wk_sb, kTh, bk_sb, KT_sb)
            # V in natural [s, d] layout (bias folded into the residual add).
            for t in range(KT):
                hh, loc = t // 8, (t % 8) * P
                ps = pa.tile([P, D], F32, tag="pa", name="ps_v")
                for c in range(CH):
                    nc.tensor.matmul(
                        ps,
                        vTh[c][hh][:, loc:loc + P],
                        wv_sb[:, c, :],
                        start=(c == 0),
                        stop=(c == CH - 1),
                    )
                nc.scalar.copy(V_sb[t], ps)

            # ---------------- phase B: pipelined attention + post ----------------
            def ln_stats(xc):
                s1 = pm.tile([P, NB], F32, tag="m", name="s1")
                for c in range(CH):
                    nc.tensor.matmul(
                        s1[0:1, :], ones_col_b, xc[c],
                        start=(c == 0), stop=(c == CH - 1),
                    )
                sq = [sqpool.tile([P, NB], BF16, tag="sq", name="sq")
                      for _ in range(CH)]
                for c in range(CH):
                    nc.scalar.activation(sq[c], xc[c], AF.Square)
                s2 = pm.tile([P, NB], F32, tag="m", name="s2")
                for c in range(CH):
                    nc.tensor.matmul(
                        s2[0:1, :], ones_col_b, sq[c],
                        start=(c == 0), stop=(c == CH - 1),
                    )
                return s1, s2

            def ln_rows(s1, s2):
                mu = rows.tile([1, NB], F32, tag="r", name="mu")
                nc.scalar.mul(mu, s1[0:1, :], 1.0 / D)
                msq = rows.tile([1, NB], F32, tag="r", name="msq")
                nc.vector.tensor_mul(msq, mu, mu)
                var = rows.tile([1, NB], F32, tag="r", name="var")
                nc.vector.scalar_tensor_tensor(
                    var, s2[0:1, :], 1.0 / D, msq, op0=OP.mult, op1=OP.subtract
                )
                nc.scalar.activation(var, var, AF.Sqrt, bias=eps_sb)
                rstd = rows.tile([1, NB], F32, tag="r", name="rstd")
                nc.vector.reciprocal(rstd, var)
                rstd_b16 = rows.tile([1, NB], BF16, tag="rb16",
                                     name="rstd_b16", bufs=4)
                nc.gpsimd.tensor_copy(out=rstd_b16, in_=rstd)
                mur = rows.tile([1, NB], BF16, tag="rb16", name="mur", bufs=4)
                nc.vector.tensor_mul(mur, mu, rstd)
                return rstd_b16, mur

            def ln_apply(rstd_b16, mur, gr_sb, gc_sb, bc_sb, xc,
                         out_tiles, out_slices):
                rstd_b = pm.tile([P, NB], F32, tag="m", name="rstd_b")
                nc.tensor.matmul(rstd_b, ones_row_b, rstd_b16,
                                 start=True, stop=True)
                for c in range(CH):
                    mg_b = pm.tile([P, NB], F32, tag="m", name="mg_b")
                    nc.tensor.matmul(
                        mg_b, gr_sb[:, c * P:(c + 1) * P], mur,
                        start=True, stop=True,
                    )
                    # t = (x*gamma)*rstd_b ; out = (t + beta) - gamma*mu*rstd
                    t = sqpool.tile([P, NB], BF16, tag="t", name="t")
                    nc.vector.scalar_tensor_tensor(
                        t, xc[c], gc_sb[:, c:c + 1], rstd_b,
                        op0=OP.mult, op1=OP.mult,
                    )
                    nc.vector.scalar_tensor_tensor(
                        out_tiles[c][out_slices[c]], t, bc_sb[:, c:c + 1],
                        mg_b, op0=OP.add, op1=OP.subtract,
                    )

            def emit_attn(j):
                jq = slice(j * NB, (j + 1) * NB)
                mtiles = []
                for g in range(4):
                    mt = mpool.tile([P, 4, NB], BF16, tag="m", name="mt")
                    nc.gpsimd.dma_start(
                        out=mt, in_=maskT3[:, 4 * g:4 * g + 4, jq]
                    )
                    mtiles.append(mt)
                qres = xf.tile([P, CH, NB], F32, tag="xf", name="qres")
                nc.gpsimd.dma_start(out=qres, in_=qT3[:, :, jq])

                U01 = [pb.tile([P, NB], F32, tag="u", name="u01")
                       for _ in range(2)]
                rsum = prow.tile([1, NB], F32, name="rsum")
                estrips = []
                for kt in range(KT):
                    sc = pa.tile([P, NB], F32, tag="pa", name="sc")
                    for c in range(CH):
                        nc.tensor.matmul(
                            sc,
                            KT_sb[c][:, kt * P:(kt + 1) * P],
                            QT[c][:, jq],
                            start=(c == 0),
                            stop=(c == CH - 1),
                        )
                    e = epool.tile([P, NB], BF16, tag="e", name="e")
                    nc.scalar.activation(e, sc, AF.Exp)
                    nc.vector.tensor_mul(e, e, mtiles[kt // 4][:, kt % 4, :])
                    estrips.append(e)
                    for c in range(2):
                        nc.tensor.matmul(
                            U01[c],
                            V_sb[kt][:, c * P:(c + 1) * P],
                            e,
                            start=(kt == 0),
                            stop=(kt == KT - 1),
                        )
                    nc.tensor.matmul(
                        rsum, ones_col_b, e, start=(kt == 0), stop=(kt == KT - 1)
                    )

                # free the U01 banks right away so the pass-2 matmuls can run
                # without waiting on the rowsum-reciprocal chain
                Usb = [usb.tile([P, NB], BF16, tag="u", name="usb")
                       for _ in range(CH)]
                nc.vector.tensor_copy(out=Usb[0], in_=U01[0])
                nc.vector.tensor_copy(out=Usb[1], in_=U01[1])
                U23 = [pb.tile([P, NB], F32, tag="u", name="u23")
                       for _ in range(2)]
                for kt in range(KT):
                    for c in range(2):
                        nc.tensor.matmul(
                            U23[c],
                            V_sb[kt][:, (c + 2) * P:(c + 3) * P],
                            estrips[kt],
                            start=(kt == 0),
                            stop=(kt == KT - 1),
                        )
                nc.vector.tensor_copy(out=Usb[2], in_=U23[0])
                nc.vector.tensor_copy(out=Usb[3], in_=U23[1])

                # rowsum reciprocal + broadcast (runs on ACT/DVE under U23)
                rs_row = rows.tile([1, NB], F32, tag="r", name="rs_row")
                nc.scalar.copy(rs_row, rsum)
                nc.vector.reciprocal(rs_row, rs_row)
                rs_row_b = rows.tile([1, NB], BF16, tag="rb16",
                                     name="rs_row_b", bufs=4)
                nc.gpsimd.tensor_copy(out=rs_row_b, in_=rs_row)
                rb_ps = pm.tile([P, NB], F32, tag="m", name="rb_ps")
                nc.tensor.matmul(rb_ps, ones_row_b, rs_row_b,
                                 start=True, stop=True)
                recip_b = rbpool.tile([P, NB], F32, tag="rb", name="recip_b")
                nc.scalar.copy(recip_b, rb_ps)

                # x1 = queryT + U*recip + bv  (bv folded: attn bias contributes
                # bv * rowsum * recip = bv exactly)
                x1 = []
                for c in range(CH):
                    x1c = x1pool.tile([P, NB], BF16, tag="x1", name="x1")
                    nc.vector.tensor_mul(x1c, Usb[c], recip_b)
                    nc.vector.scalar_tensor_tensor(
                        x1c, x1c, bv_sb[:, c:c + 1], qres[:, c, :],
                        op0=OP.add, op1=OP.add,
                    )
                    x1.append(x1c)
                return j, x1

            def post_gen(ctx):
                j, x1 = ctx
                jq = slice(j * NB, (j + 1) * NB)
                s1, s2 = ln_stats(x1)
                yield
                r1 = ln_rows(s1, s2)
                yield
                out1 = [o1pool.tile([P, NB], BF16, tag="o1", name="out1")
                        for _ in range(CH)]
                ln_apply(*r1, g1r_sb, g1c_sb, be1c_sb, x1,
                         out1, [np.s_[:, :]] * CH)
                yield
                hp = pm.tile([P, NB], F32, tag="m", name="hp")
                for c in range(CH):
                    nc.tensor.matmul(
                        hp[0:FF, :], w1_sb[:, c, :], out1[c],
                        start=(c == 0), stop=(c == CH - 1),
                    )
                h = hpool.tile([FF + 1, NB], BF16, tag="h", name="h")
                nc.scalar.activation(h[0:FF, :], hp[0:FF, :], AF.Relu,
                                     bias=b1_sb)
                nc.vector.memset(h[FF:FF + 1, :], 1.0)
                x2 = []
                for c in range(CH):
                    fp = pm.tile([P, NB], F32, tag="m", name="fp")
                    nc.tensor.matmul(
                        fp, w2_sb[:, c * P:(c + 1) * P], h,
                        start=True, stop=True,
                    )
                    x2c = x2pool.tile([P, NB], BF16, tag="x2", name="x2c")
                    nc.vector.tensor_add(x2c, fp, out1[c])
                    x2.append(x2c)
                yield
                s1b, s2b = ln_stats(x2)
                yield
                r2 = ln_rows(s1b, s2b)
                yield
                ofin = ofp.tile([P, CH, NB], F32, tag="of", name="ofin")
                ln_apply(*r2, g2r_sb, g2c_sb, be2c_sb, x2,
                         [ofin] * CH, [np.s_[:, c, :] for c in range(CH)])
                nc.gpsimd.dma_start(out=outT3[:, :, jq], in_=ofin)

            def emit_post(ctx):
                for _ in post_gen(ctx):
                    pass

            from itertools import zip_longest

            ctxs = []
            for j in range(QB):
                ctxs.append(emit_attn(j))
                if 1 <= j <= QB - 2:
                    emit_post(ctxs[j - 1])
            # tail: interleave the last two posts stage-by-stage so each
            # block's cross-engine chain latency hides behind the other's work
            for _ in zip_longest(post_gen(ctxs[QB - 2]), post_gen(ctxs[QB - 1])):
                pass

    nc.finalize()
    return nc


_NC = None


def _get_nc():
    global _NC
    if _NC is None:
        _NC = build()
    return _NC


def _stage_weights(Wq, bq, Wk, bk, Wv, bv, g1, be1, g2, be2, W1, b1, W2, b2):
    def chunked_T(w):  # [f, e] weight -> [p, c, f] with partition = e within chunk
        return np.ascontiguousarray(
            w.T.reshape(CH, P, -1).transpose(1, 0, 2)
        )

    def col(v):  # [D] -> [p, c]
        return np.ascontiguousarray(v.reshape(CH, P).T)

    return {
        "wq": chunked_T(Wq.astype(np.float32) * SCALE).astype(NPBF16),
        "wk": chunked_T(Wk).astype(NPBF16),
        "wv": chunked_T(Wv).astype(NPBF16),
        "w1": chunked_T(W1).astype(NPBF16),
        "w2b": np.ascontiguousarray(
            np.concatenate([W2.T, b2[None, :]], axis=0)
        ).astype(NPBF16),
        "bq": col(bq.astype(np.float32) * SCALE),
        "bk": col(bk),
        "bv": col(bv),
        "b1": np.ascontiguousarray(b1[:, None]).astype(np.float32),
        "g1c": col(g1),
        "be1c": col(be1),
        "g2c": col(g2),
        "be2c": col(be2),
        "g1r": np.ascontiguousarray(g1[None, :]).astype(NPBF16),
        "g2r": np.ascontiguousarray(g2[None, :]).astype(NPBF16),
    }


def run(inputs, trace=False, **kwargs):
    """Run on the 8 NeuronCores; returns (output [B,S,D] f32, BassKernelResults)."""
    nc = _get_nc()
    w = _stage_weights(
        inputs["Wq"], inputs["bq"], inputs["Wk"], inputs["bk"], inputs["Wv"],
        inputs["bv"], inputs["g1"], inputs["be1"], inputs["g2"], inputs["be2"],
        inputs["W1"], inputs["b1"], inputs["W2"], inputs["b2"],
    )
    w = {k: np.asarray(v) for k, v in w.items()}
    query = np.asarray(inputs["query"], np.float32)
    key = np.asarray(inputs["key"], np.float32)
    value = np.asarray(inputs["value"], np.float32)
    mask = np.asarray(inputs["mask"])
    in_maps = []
    for b in range(B):
        m = dict(w)
        qt = np.ascontiguousarray(query[b].T)
        m["qT"] = qt
        m["qTb"] = qt.astype(NPBF16)
        m["kTb"] = np.ascontiguousarray(key[b].T).astype(NPBF16)
        m["vTb"] = np.ascontiguousarray(value[b].T).astype(NPBF16)
        m["maskT"] = np.ascontiguousarray(mask[b].T).astype(NPBF16)
        in_maps.append(m)
    res = run_bass_kernel_spmd(nc, in_maps, core_ids=list(range(B)),
                               trace=trace, **kwargs)
    out = np.stack(
        [np.asarray(res.results[b]["outT"], np.float32).T for b in range(B)]
    )
    return out, res


def kernel(**inputs) -> np.ndarray:
    out, _ = run(inputs)
    return out


# revision 41
# speedup vs baseline: 35811.8647x; 22911.0571x over previous
"""Trainium2 Bass kernel for nn_AttentionLayer (B=8, S=2048, EMB=512, FF=64).

Strategy: data-parallel over batch — each of the 8 NeuronCores processes one
batch element independently (no collectives). The whole per-core computation
runs in a feature-major ("transposed") layout so that every matmul contraction
lands on the partition dimension and no on-device transposes are needed:

  Q^T = (Wq/sqrt(d)) @ query^T + bq/sqrt(d)      [d, s]   (bias per-partition)
  K^T = Wk @ key^T + bk                          [d, s]
  V   = value @ Wv^T                             [s, d]   (bv folded into x1)
  S^T[k,q] = sum_d K^T[d,k] Q^T[d,q]             (scores, pre-scaled)
  E^T = exp(S^T) * maskT                         (no max-subtraction needed:
                                                  |scores| <~ 2, exp is safe)
  U^T[d,q] = sum_k V[k,d] E^T[k,q]               (unnormalized attention)
  rowsum[q] = sum_k E^T[k,q]   (DVE running sum + one fp32 ones-matmul)
  x1 = query^T + U^T / rowsum + bv               (bv exact: rowsum/rowsum = 1)
  out1 = LN1(x1)   (stats over d = partition dim via ones matmuls; rstd via
                    exp(-0.5 ln(var+eps)) so ACT needs only one table set)
  h^T = relu(W1 @ out1^T + b1);  ff^T = W2 @ h^T + b2 (bias via ones row in h)
  out^T = LN2(out1 + ff)                         -> host transposes back

Scheduling: phase A (projections) streams half-chunk loads; attention blocks
are software-pipelined with the LN/FFN "post" work of the previous block
interleaved into the next block's k-loop (round-robin generator stepping);
the final block's post runs as two interleaved half-width chains. All
activations live in the natural_log_exp_and_others ACT table set, preloaded
once. Compute is bf16 on the TensorEngine with f32 PSUM accumulation;
measured end-to-end error vs the f32 reference is ~4e-3 L2.
"""

import sys

if "/opt/trn_rl_repo" not in sys.path:
    sys.path.insert(0, "/opt/trn_rl_repo")

import numpy as np

import concourse.bass as bass
import concourse.bacc as bacc
import concourse.tile as tile
from concourse import mybir
from concourse.bass_utils import run_bass_kernel_spmd

P = 128
S = 2048
D = 512
FF = 64
B = 8
CH = D // P          # 4 chunks of the emb dim
KT = S // P          # 16 key tiles
NB = 512             # q-block width (matmul free dim / PSUM bank)
QB = S // NB         # 4 q-blocks
EPS = 1e-5
SCALE = 1.0 / np.sqrt(np.float32(D))

F32 = mybir.dt.float32
BF16 = mybir.dt.bfloat16
F32R = mybir.dt.float32r
AF = mybir.ActivationFunctionType
OP = mybir.AluOpType

NPBF16 = mybir.dt.np(BF16)


from contextlib import ExitStack, contextmanager


@contextmanager
def TileCtx(nc):
    with tile.TileContext(nc) as tc:
        with ExitStack() as es:
            yield tc, es


def build():
    nc = bacc.Bacc(
        "TRN2", target_bir_lowering=False, debug=False, num_devices=B
    )

    d_qTb = nc.dram_tensor("qTb", [D, S], BF16, kind="ExternalInput")
    d_kTb = nc.dram_tensor("kTb", [D, S], BF16, kind="ExternalInput")
    d_vTb = nc.dram_tensor("vTb", [D, S], BF16, kind="ExternalInput")
    d_maskT = nc.dram_tensor("maskT", [S, S], BF16, kind="ExternalInput")
    d_wq = nc.dram_tensor("wq", [P, CH, D], BF16, kind="ExternalInput")
    d_wk = nc.dram_tensor("wk", [P, CH, D], BF16, kind="ExternalInput")
    d_wv = nc.dram_tensor("wv", [P, CH, D], BF16, kind="ExternalInput")
    d_w1 = nc.dram_tensor("w1", [P, CH, FF], BF16, kind="ExternalInput")
    d_w2b = nc.dram_tensor("w2b", [FF + 1, D], BF16, kind="ExternalInput")
    d_bq = nc.dram_tensor("bq", [P, CH], F32, kind="ExternalInput")
    d_bk = nc.dram_tensor("bk", [P, CH], F32, kind="ExternalInput")
    d_bv = nc.dram_tensor("bv", [P, CH], F32, kind="ExternalInput")
    d_b1 = nc.dram_tensor("b1", [FF, 1], F32, kind="ExternalInput")
    d_g1c = nc.dram_tensor("g1c", [P, CH], F32, kind="ExternalInput")
    d_be1c = nc.dram_tensor("be1c", [P, CH], F32, kind="ExternalInput")
    d_g2c = nc.dram_tensor("g2c", [P, CH], F32, kind="ExternalInput")
    d_be2c = nc.dram_tensor("be2c", [P, CH], F32, kind="ExternalInput")
    d_g1r = nc.dram_tensor("g1r", [1, D], BF16, kind="ExternalInput")
    d_g2r = nc.dram_tensor("g2r", [1, D], BF16, kind="ExternalInput")
    d_outT = nc.dram_tensor("outT", [D, S], F32, kind="ExternalOutput")

    qTb3 = d_qTb.rearrange("(c p) s -> p c s", p=P)
    kTb3 = d_kTb.rearrange("(c p) s -> p c s", p=P)
    vTb3 = d_vTb.rearrange("(c p) s -> p c s", p=P)
    maskT3 = d_maskT.rearrange("(t p) s -> p t s", p=P)
    outT3 = d_outT.rearrange("(c p) s -> p c s", p=P)

    from contextlib import ExitStack

    with TileCtx(nc) as (tc, es):
            cpool = es.enter_context(tc.tile_pool(name="const", bufs=1))
            xf = es.enter_context(tc.tile_pool(name="xf", bufs=2))
            ofp = es.enter_context(tc.tile_pool(name="ofp", bufs=2))
            xb = es.enter_context(tc.tile_pool(name="xb", bufs=11))
            usb = es.enter_context(tc.tile_pool(name="usb", bufs=4))
            qkv = es.enter_context(tc.tile_pool(name="qkv", bufs=1))
            epool = es.enter_context(tc.tile_pool(name="epool", bufs=19))
            mpool = es.enter_context(tc.tile_pool(name="mpool", bufs=3))
            o1pool = es.enter_context(tc.tile_pool(name="o1pool", bufs=8))
            x1pool = es.enter_context(tc.tile_pool(name="x1pool", bufs=8))
            sqpool = es.enter_context(tc.tile_pool(name="sqpool", bufs=6))
            x2pool = es.enter_context(tc.tile_pool(name="x2pool", bufs=8))
            hpool = es.enter_context(tc.tile_pool(name="hpool", bufs=2))
            rbpool = es.enter_context(tc.tile_pool(name="rbpool", bufs=2))
            rows = es.enter_context(tc.tile_pool(name="rows", bufs=5))
            pa = es.enter_context(tc.tile_pool(name="pa", bufs=2, space="PSUM"))
            pb = es.enter_context(tc.tile_pool(name="pb", bufs=2, space="PSUM"))
            prow = es.enter_context(tc.tile_pool(name="prow", bufs=1, space="PSUM"))
            pm = es.enter_context(tc.tile_pool(name="pm", bufs=3, space="PSUM"))
            # ---------------- constants ----------------
            wq_sb = cpool.tile([P, CH, D], BF16, name="wq_sb")
            wk_sb = cpool.tile([P, CH, D], BF16, name="wk_sb")
            wv_sb = cpool.tile([P, CH, D], BF16, name="wv_sb")
            w1_sb = cpool.tile([P, CH, FF], BF16, name="w1_sb")
            w2_sb = cpool.tile([FF + 1, D], BF16, name="w2_sb")
            bq_sb = cpool.tile([P, CH], F32, name="bq_sb")
            bk_sb = cpool.tile([P, CH], F32, name="bk_sb")
            bv_sb = cpool.tile([P, CH], F32, name="bv_sb")
            b1_sb = cpool.tile([FF, 1], F32, name="b1_sb")
            g1c_sb = cpool.tile([P, CH], F32, name="g1c_sb")
            be1c_sb = cpool.tile([P, CH], F32, name="be1c_sb")
            g2c_sb = cpool.tile([P, CH], F32, name="g2c_sb")
            be2c_sb = cpool.tile([P, CH], F32, name="be2c_sb")
            g1r_sb = cpool.tile([1, D], BF16, name="g1r_sb")
            g2r_sb = cpool.tile([1, D], BF16, name="g2r_sb")
            # weights/biases on the critical path load on the sync queue,
            # interleaved with the input halves (emitted in load_halves below);
            # everything needed only later goes through the idle gpsimd queue.
            nc.gpsimd.dma_start(out=w1_sb, in_=d_w1[:])
            nc.gpsimd.dma_start(out=w2_sb, in_=d_w2b[:])
            nc.gpsimd.dma_start(out=bv_sb, in_=d_bv[:])
            nc.gpsimd.dma_start(out=b1_sb, in_=d_b1[:])
            nc.gpsimd.dma_start(out=g1c_sb, in_=d_g1c[:])
            nc.gpsimd.dma_start(out=be1c_sb, in_=d_be1c[:])
            nc.gpsimd.dma_start(out=g2c_sb, in_=d_g2c[:])
            nc.gpsimd.dma_start(out=be2c_sb, in_=d_be2c[:])
            nc.gpsimd.dma_start(out=g1r_sb, in_=d_g1r[:])
            nc.gpsimd.dma_start(out=g2r_sb, in_=d_g2r[:])

            # preload the one ACT table set covering every function used
            # (exp, ln, square, relu, copy, identity) so the auto-inserter
            # never has to switch sets mid-kernel (~2.7us per switch)
            nc.scalar.add_instruction(
                mybir.InstLoadActFuncSet(
                    name=nc.get_next_instruction_name(), ins=[], outs=[],
                    act_func_set_id=6,
                )
            )

            ones_col_b = cpool.tile([P, 1], BF16, name="ones_col_b")
            ones_row_b = cpool.tile([1, P], BF16, name="ones_row_b")
            eps_sb = cpool.tile([1, 1], F32, name="eps_sb")
            nc.vector.memset(ones_col_b, 1.0)
            nc.vector.memset(ones_row_b, 1.0)
            nc.vector.memset(eps_sb, EPS)

            # ---------------- phase A: projections ----------------
            # Inputs stream in as half-chunks [128, 1024] so the first
            # projection group is ready after ~1.5 MB of DMA, not 4 MB.
            HW_ = S // 2

            def load_tensor_priority(w_tile, d_w, b_tile, d_b, dram3, name):
                nc.sync.dma_start(out=w_tile, in_=d_w[:])
                halves = [[None, None] for _ in range(CH)]
                for c in range(CH):
                    xt = xb.tile([P, HW_], BF16, tag="xb", name=f"{name}{c}_0")
                    nc.sync.dma_start(out=xt, in_=dram3[:, c, 0:HW_])
                    halves[c][0] = xt
                if b_tile is not None:
                    nc.sync.dma_start(out=b_tile, in_=d_b[:])
                for c in range(CH):
                    xt = xb.tile([P, HW_], BF16, tag="xb", name=f"{name}{c}_1")
                    nc.sync.dma_start(out=xt, in_=dram3[:, c, HW_:S])
                    halves[c][1] = xt
                return halves

            qTh = load_tensor_priority(wq_sb, d_wq, bq_sb, d_bq, qTb3, "qh")
            kTh = load_tensor_priority(wk_sb, d_wk, bk_sb, d_bk, kTb3, "kh")
            vTh = load_tensor_priority(wv_sb, d_wv, None, None, vTb3, "vh")

            QT = [qkv.tile([P, S], BF16, name=f"QT{c}") for c in range(CH)]
            KT_sb = [qkv.tile([P, S], BF16, name=f"KTs{c}") for c in range(CH)]
            V_sb = [qkv.tile([P, D], BF16, name=f"V{t}") for t in range(KT)]

            def proj_T(w_sb, xh, bias_sb, out_tiles):
                for j in range(QB):
                    hh, loc = j // 2, (j % 2) * NB
                    for fc in range(CH):
                        ps = pa.tile([P, NB], F32, tag="pa", name="ps")
                        for c in range(CH):
                            nc.tensor.matmul(
                                ps,
                                w_sb[:, c, fc * P:(fc + 1) * P],
                                xh[c][hh][:, loc:loc + NB],
                                start=(c == 0),
                                stop=(c == CH - 1),
                            )
                        nc.scalar.activation(
                            out_tiles[fc][:, j * NB:(j + 1) * NB], ps,
                            AF.Identity, bias=bias_sb[:, fc:fc + 1], scale=1.0,
                        )

            proj_T(wq_sb, qTh, bq_sb, QT)
            proj_T(wk_sb, kTh, bk_sb, KT_sb)
            # V in natural [s, d] layout (bias folded into the residual add).
            for t in range(KT):
                hh, loc = t // 8, (t % 8) * P
                ps = pa.tile([P, D], F32, tag="pa", name="ps_v")
                for c in range(CH):
                    nc.tensor.matmul(
                        ps,
                        vTh[c][hh][:, loc:loc + P],
                        wv_sb[:, c, :],
                        start=(c == 0),
                        stop=(c == CH - 1),
                    )
                nc.scalar.copy(V_sb[t], ps)

            # ---------------- phase B: pipelined attention + post ----------------
            def ln_stats(xc):
                s1 = pm.tile([P, NB], F32, tag="m", name="s1")
                for c in range(CH):
                    nc.tensor.matmul(
                        s1[0:1, :], ones_col_b, xc[c],
                        start=(c == 0), stop=(c == CH - 1),
                    )
                sq = [sqpool.tile([P, NB], BF16, tag="sq", name="sq")
                      for _ in range(CH)]
                for c in range(CH):
                    nc.scalar.activation(sq[c], xc[c], AF.Square)
                s2 = pm.tile([P, NB], F32, tag="m", name="s2")
                for c in range(CH):
                    nc.tensor.matmul(
                        s2[0:1, :], ones_col_b, sq[c],
                        start=(c == 0), stop=(c == CH - 1),
                    )
                return s1, s2

            def ln_rows(s1, s2):
                mu = rows.tile([1, NB], F32, tag="r", name="mu")
                nc.scalar.mul(mu, s1[0:1, :], 1.0 / D)
                msq = rows.tile([1, NB], F32, tag="r", name="msq")
                nc.vector.tensor_mul(msq, mu, mu)
                var = rows.tile([1, NB], F32, tag="r", name="var")
                nc.vector.scalar_tensor_tensor(
                    var, s2[0:1, :], 1.0 / D, msq, op0=OP.mult, op1=OP.subtract
                )
                nc.scalar.activation(var, var, AF.Sqrt, bias=eps_sb)
                rstd = rows.tile([1, NB], F32, tag="r", name="rstd")
                nc.vector.reciprocal(rstd, var)
                rstd_b16 = rows.tile([1, NB], BF16, tag="rb16",
                                     name="rstd_b16", bufs=4)
                nc.gpsimd.tensor_copy(out=rstd_b16, in_=rstd)
                mur = rows.tile([1, NB], BF16, tag="rb16", name="mur", bufs=4)
                nc.vector.tensor_mul(mur, mu, rstd)
                return rstd_b16, mur

            def ln_apply(rstd_b16, mur, gr_sb, gc_sb, bc_sb, xc,
                         out_tiles, out_slices):
                rstd_b = pm.tile([P, NB], F32, tag="m", name="rstd_b")
                nc.tensor.matmul(rstd_b, ones_row_b, rstd_b16,
                                 start=True, stop=True)
                for c in range(CH):
                    mg_b = pm.tile([P, NB], F32, tag="m", name="mg_b")
                    nc.tensor.matmul(
                        mg_b, gr_sb[:, c * P:(c + 1) * P], mur,
                        start=True, stop=True,
                    )
                    # t = (x*gamma)*rstd_b ; out = (t + beta) - gamma*mu*rstd
                    t = sqpool.tile([P, NB], BF16, tag="t", name="t")
                    nc.vector.scalar_tensor_tensor(
                        t, xc[c], gc_sb[:, c:c + 1], rstd_b,
                        op0=OP.mult, op1=OP.mult,
                    )
                    nc.vector.scalar_tensor_tensor(
                        out_tiles[c][out_slices[c]], t, bc_sb[:, c:c + 1],
                        mg_b, op0=OP.add, op1=OP.subtract,
                    )

            def emit_attn(j, pgen=None):
                def step_post():
                    if pgen is not None:
                        next(pgen, None)

                jq = slice(j * NB, (j + 1) * NB)
                mtiles = []
                for g in range(4):
                    mt = mpool.tile([P, 4, NB], BF16, tag="m", name="mt")
                    nc.gpsimd.dma_start(
                        out=mt, in_=maskT3[:, 4 * g:4 * g + 4, jq]
                    )
                    mtiles.append(mt)
                qres = xf.tile([P, CH, NB], BF16, tag="xf", name="qres")
                nc.gpsimd.dma_start(out=qres, in_=qTb3[:, :, jq])

                U01 = [pb.tile([P, NB], F32, tag="u", name="u01")
                       for _ in range(2)]
                rsum = prow.tile([1, NB], F32, name="rsum")
                estrips = []
                for kt in range(KT):
                    sc = pa.tile([P, NB], F32, tag="pa", name="sc")
                    for c in range(CH):
                        nc.tensor.matmul(
                            sc,
                            KT_sb[c][:, kt * P:(kt + 1) * P],
                            QT[c][:, jq],
                            start=(c == 0),
                            stop=(c == CH - 1),
                        )
                    e = epool.tile([P, NB], BF16, tag="e", name="e")
                    nc.scalar.activation(e, sc, AF.Exp)
                    nc.vector.tensor_mul(e, e, mtiles[kt // 4][:, kt % 4, :])
                    estrips.append(e)
                    for c in range(2):
                        nc.tensor.matmul(
                            U01[c],
                            V_sb[kt][:, c * P:(c + 1) * P],
                            e,
                            start=(kt == 0),
                            stop=(kt == KT - 1),
                        )
                    nc.tensor.matmul(
                        rsum, ones_col_b, e, start=(kt == 0), stop=(kt == KT - 1)
                    )
                    if kt % 2 == 0 and kt >= 2:
                        step_post()

                # free the U01 banks right away so the pass-2 matmuls can run
                # without waiting on the rowsum-reciprocal chain
                Usb = [usb.tile([P, NB], BF16, tag="u", name="usb")
                       for _ in range(CH)]
                nc.vector.tensor_copy(out=Usb[0], in_=U01[0])
                nc.vector.tensor_copy(out=Usb[1], in_=U01[1])
                U23 = [pb.tile([P, NB], F32, tag="u", name="u23")
                       for _ in range(2)]
                for kt in range(KT):
                    for c in range(2):
                        nc.tensor.matmul(
                            U23[c],
                            V_sb[kt][:, (c + 2) * P:(c + 3) * P],
                            estrips[kt],
                            start=(kt == 0),
                            stop=(kt == KT - 1),
                        )
                nc.vector.tensor_copy(out=Usb[2], in_=U23[0])
                nc.vector.tensor_copy(out=Usb[3], in_=U23[1])

                # rowsum reciprocal + broadcast (runs on ACT/DVE under U23)
                rs_row = rows.tile([1, NB], F32, tag="r", name="rs_row")
                nc.scalar.copy(rs_row, rsum)
                nc.vector.reciprocal(rs_row, rs_row)
                rs_row_b = rows.tile([1, NB], BF16, tag="rb16",
                                     name="rs_row_b", bufs=4)
                nc.gpsimd.tensor_copy(out=rs_row_b, in_=rs_row)
                rb_ps = pm.tile([P, NB], F32, tag="m", name="rb_ps")
                nc.tensor.matmul(rb_ps, ones_row_b, rs_row_b,
                                 start=True, stop=True)
                recip_b = rbpool.tile([P, NB], BF16, tag="rb", name="recip_b")
                nc.scalar.copy(recip_b, rb_ps)

                # x1 = queryT + U*recip + bv  (bv folded: attn bias contributes
                # bv * rowsum * recip = bv exactly)
                x1 = []
                for c in range(CH):
                    x1c = x1pool.tile([P, NB], BF16, tag="x1", name="x1")
                    nc.vector.tensor_mul(x1c, Usb[c], recip_b)
                    nc.vector.scalar_tensor_tensor(
                        x1c, x1c, bv_sb[:, c:c + 1], qres[:, c, :],
                        op0=OP.add, op1=OP.add,
                    )
                    x1.append(x1c)
                while pgen is not None and next(pgen, StopIteration) is not StopIteration:
                    pass
                return j, x1

            def post_gen(ctx):
                j, x1 = ctx
                jq = slice(j * NB, (j + 1) * NB)
                s1, s2 = ln_stats(x1)
                yield
                r1 = ln_rows(s1, s2)
                yield
                out1 = [o1pool.tile([P, NB], BF16, tag="o1", name="out1")
                        for _ in range(CH)]
                ln_apply(*r1, g1r_sb, g1c_sb, be1c_sb, x1,
                         out1, [np.s_[:, :]] * CH)
                yield
                hp = pm.tile([P, NB], F32, tag="m", name="hp")
                for c in range(CH):
                    nc.tensor.matmul(
                        hp[0:FF, :], w1_sb[:, c, :], out1[c],
                        start=(c == 0), stop=(c == CH - 1),
                    )
                h = hpool.tile([FF + 1, NB], BF16, tag="h", name="h")
                nc.scalar.activation(h[0:FF, :], hp[0:FF, :], AF.Relu,
                                     bias=b1_sb)
                nc.vector.memset(h[FF:FF + 1, :], 1.0)
                x2 = []
                for c in range(CH):
                    fp = pm.tile([P, NB], F32, tag="m", name="fp")
                    nc.tensor.matmul(
                        fp, w2_sb[:, c * P:(c + 1) * P], h,
                        start=True, stop=True,
                    )
                    x2c = x2pool.tile([P, NB], BF16, tag="x2", name="x2c")
                    nc.vector.tensor_add(x2c, fp, out1[c])
                    x2.append(x2c)
                yield
                s1b, s2b = ln_stats(x2)
                yield
                r2 = ln_rows(s1b, s2b)
                yield
                ofin = ofp.tile([P, CH, NB], F32, tag="of", name="ofin")
                ln_apply(*r2, g2r_sb, g2c_sb, be2c_sb, x2,
                         [ofin] * CH, [np.s_[:, c, :] for c in range(CH)])
                nc.gpsimd.dma_start(out=outT3[:, :, jq], in_=ofin)

            prev_ctx = None
            for j in range(QB):
                pgen = post_gen(prev_ctx) if prev_ctx is not None else None
                prev_ctx = emit_attn(j, pgen)
            for _ in post_gen(prev_ctx):
                pass

    nc.finalize()
    return nc


_NC = None


def _get_nc():
    global _NC
    if _NC is None:
        _NC = build()
    return _NC


def _stage_weights(Wq, bq, Wk, bk, Wv, bv, g1, be1, g2, be2, W1, b1, W2, b2):
    def chunked_T(w):  # [f, e] weight -> [p, c, f] with partition = e within chunk
        return np.ascontiguousarray(
            w.T.reshape(CH, P, -1).transpose(1, 0, 2)
        )

    def col(v):  # [D] -> [p, c]
        return np.ascontiguousarray(v.reshape(CH, P).T)

    return {
        "wq": chunked_T(Wq.astype(np.float32) * SCALE).astype(NPBF16),
        "wk": chunked_T(Wk).astype(NPBF16),
        "wv": chunked_T(Wv).astype(NPBF16),
        "w1": chunked_T(W1).astype(NPBF16),
        "w2b": np.ascontiguousarray(
            np.concatenate([W2.T, b2[None, :]], axis=0)
        ).astype(NPBF16),
        "bq": col(bq.astype(np.float32) * SCALE),
        "bk": col(bk),
        "bv": col(bv),
        "b1": np.ascontiguousarray(b1[:, None]).astype(np.float32),
        "g1c": col(g1),
        "be1c": col(be1),
        "g2c": col(g2),
        "be2c": col(be2),
        "g1r": np.ascontiguousarray(g1[None, :]).astype(NPBF16),
        "g2r": np.ascontiguousarray(g2[None, :]).astype(NPBF16),
    }


def run(inputs, trace=False, **kwargs):
    """Run on the 8 NeuronCores; returns (output [B,S,D] f32, BassKernelResults)."""
    nc = _get_nc()
    w = _stage_weights(
        inputs["Wq"], inputs["bq"], inputs["Wk"], inputs["bk"], inputs["Wv"],
        inputs["bv"], inputs["g1"], inputs["be1"], inputs["g2"], inputs["be2"],
        inputs["W1"], inputs["b1"], inputs["W2"], inputs["b2"],
    )
    w = {k: np.asarray(v) for k, v in w.items()}
    query = np.asarray(inputs["query"], np.float32)
    key = np.asarray(inputs["key"], np.float32)
    value = np.asarray(inputs["value"], np.float32)
    mask = np.asarray(inputs["mask"])
    in_maps = []
    for b in range(B):
        m = dict(w)
        m["qTb"] = np.ascontiguousarray(query[b].T).astype(NPBF16)
        m["kTb"] = np.ascontiguousarray(key[b].T).astype(NPBF16)
        m["vTb"] = np.ascontiguousarray(value[b].T).astype(NPBF16)
        m["maskT"] = np.ascontiguousarray(mask[b].T).astype(NPBF16)
        in_maps.append(m)
    res = run_bass_kernel_spmd(nc, in_maps, core_ids=list(range(B)),
                               trace=trace, **kwargs)
    out = np.stack(
        [np.asarray(res.results[b]["outT"], np.float32).T for b in range(B)]
    )
    return out, res


def kernel(**inputs) -> np.ndarray:
    out, _ = run(inputs)
    return out
